# revision 28
# baseline (speedup 1.0000x reference)
"""DMVFlow per-state diagonal-Gaussian log-density kernel for 8 TRN2 NeuronCores.

density[b,t,k] = log_norm - 0.5*(s2[b,t] - 2*cross[b,t,k] + m2[k])
  with  log_norm = -0.5*(D*log(2pi) + sum_d log var[d])
        s2[b,t]  = sum_d s[b,t,d]^2 / var[d]
        cross    = sum_d s[b,t,d] * means[k,d] / var[d]
        m2[k]    = sum_d means[k,d]^2 / var[d]

Only cross[b,t,k] couples (b,t) with k; the per-row term (log_norm - 0.5*s2)
and per-state term (-0.5*m2) are rank-1 in the output and are computed exactly
on the host and added during assembly.  The device therefore runs a single
fp8(e4m3) GEMM per core: cross = s @ (means/var).T, using DoubleRow perf mode
(two 128-deep k-tiles per instruction).

Sharding: data-parallel over batch (32 sentences per core), weights replicated.

The kernel is input-DMA-bound (6.3 MB fp8 per core over 16 DMA engines at
~23 GB/s each).  Input arrives as st[p, t, c, n] = fp8(s[row = t*512 + n,
d = c*128 + p]) so DMA lines are contiguous per partition; tiles stream in
chunks of <=2 tiles (6KB packets -- 12KB packets measured ~20% slower/byte)
alternating across the sync and gpsimd queues so descriptor spin-up gaps on
one queue hide under the other queue's stream.  Device output is int8
(cross/2, |x| <= 118) to halve store traffic; host rescales and adds the
affine terms in fp32.  PSUM->int8 casts alternate DVE/ACT by tile parity.
"""

import numpy as np

N_CORES = 8
B, T, D, K = 256, 256, 768, 128
BPC = B // N_CORES          # batches per core
R = BPC * T                 # rows (token positions) per core = 8192
TN = 512                    # rows per tile (one PSUM bank)
NT = R // TN                # tiles per core = 16
C = D // 128                # contraction chunks = 6
G = C // 2                  # DoubleRow double-chunks = 3

OSCALE = 2.0                # host multiplier undoing the device's 0.5

# input DMA tiles per queue: 1-tile DMAs = 3KB packets, the measured
# per-DMA-engine sweet spot (21.5 GB/s; 6KB packets drop to 18, 12KB to
# 17); descriptor gaps on one queue hide under the other queue's stream.
# The two queues drain proportionally, so with a balanced split the last
# tiles of BOTH queues land bunched and the in-order PE serializes a
# 2-3 tile backlog.  Instead the scalar queue gets 7 tiles and drains
# early; sync then delivers 13,14,15 alone at full engine rate (~1.2us
# apart), which the PE consumes as they land.
SYNC_TILES = [0, 2, 4, 6, 8, 10, 12, 13, 14, 15]
SCAL_TILES = [1, 3, 5, 7, 9, 11]

# quad stores + even final tiles on gpsimd (DVE cannot issue DMAs);
# scalar stores odd final tiles 13/15 itself right after casting them,
# so the critical last store has zero cross-engine latency
STORES_GPS = [(0, 4), (4, 8), (8, 12), (12, 13), (14, 15)]
N_STORES = len(STORES_GPS) + 2

_NC = None                  # cached bass program (build once per process)


def _build_nc_fp8():
    """Hand-scheduled fp8 DoubleRow kernel: no TileContext, manual semaphores.

    Engine roles:
      sync   - weights DMA + even input tiles + tile 15 (HW-DGE queue)
      scalar - odd input tiles + tile 14 (HW-DGE queue), odd casts (ACT)
      gpsimd - output stores (own queue so they don't FIFO behind input)
      vector - even-tile PSUM casts (DVE)
      tensor - 3 DoubleRow matmuls per tile
    """
    from contextlib import ExitStack

    import concourse.bacc as bacc
    from concourse import mybir

    f8 = mybir.dt.float8e4
    i8 = mybir.dt.int8
    f32 = mybir.dt.float32
    DR = mybir.MatmulPerfMode.DoubleRow

    NPS = 8      # psum banks

    nc = bacc.Bacc(None, target_bir_lowering=False, debug=False)

    st = nc.dram_tensor("st", [128, NT, C, TN], f8, kind="ExternalInput")
    wv = nc.dram_tensor("wv", [128, C, K], f8, kind="ExternalInput")
    out = nc.dram_tensor("out", [K, R], i8, kind="ExternalOutput")

    with ExitStack() as ctx:
        e = ctx.enter_context
        s_sb = e(nc.sbuf_tensor([128, NT, C, TN], f8))
        o_sb = e(nc.sbuf_tensor([K, NT, TN], i8))
        wv_sb = e(nc.sbuf_tensor([128, C, K], f8))
        ps = [e(nc.psum_tensor(f"ps{i}", [K, TN], f32)) for i in range(NPS)]

        in_sems = [e(nc.semaphore(f"in{j}")) for j in range(NT)]
        wv_sem = e(nc.semaphore("wv_sem"))      # +16 when weights resident
        pe_sem = e(nc.semaphore("pe_sem"))      # +1 per finished MM group
        cast_sems = [e(nc.semaphore("castE")), e(nc.semaphore("castO"))]
        out_sem = e(nc.semaphore("out_sem"))    # +16 per completed store
        blk = e(nc.Block())

        def cast_wait(eng, lo, hi):
            # casts of tiles lo..hi-1 complete (per-parity counters)
            if hi - lo == 1:
                eng.wait_ge(cast_sems[lo % 2], lo // 2 + 1)
            else:
                eng.wait_ge(cast_sems[0], (hi + 1) // 2)
                eng.wait_ge(cast_sems[1], hi // 2)

        def issue_tiles(eng, tiles):
            for t in tiles:
                eng.dma_start(
                    s_sb[:, t, :, :], st[:, t, :, :]
                ).then_inc(in_sems[t], 16)

        @blk.sync
        def _(eng):
            eng.dma_start(wv_sb[:], wv[:]).then_inc(wv_sem, 16)
            issue_tiles(eng, SYNC_TILES)

        def store_tile(eng, t):
            eng.dma_start(
                out[:, t * TN : (t + 1) * TN], o_sb[:, t, :]
            ).then_inc(out_sem, 16)

        @blk.scalar
        def _(eng):
            issue_tiles(eng, SCAL_TILES)
            for t in range(1, NT, 2):
                eng.wait_ge(pe_sem, t + 1)
                nc.scalar.mul(o_sb[:, t, :], ps[t % NPS][:], 0.5).then_inc(
                    cast_sems[1], 1
                )
                if t >= 12:
                    store_tile(eng, t)

        @blk.gpsimd
        def _(eng):
            for lo, hi in STORES_GPS:
                cast_wait(eng, lo, hi)
                eng.dma_start(
                    out[:, lo * TN : hi * TN], o_sb[:, lo:hi, :]
                ).then_inc(out_sem, 16)
            eng.wait_ge(out_sem, 16 * N_STORES)

        @blk.vector
        def _(eng):
            for t in range(0, NT, 2):
                eng.wait_ge(pe_sem, t + 1)
                nc.vector.tensor_scalar_mul(
                    o_sb[:, t, :], ps[t % NPS][:], 0.5
                ).then_inc(cast_sems[0], 1)

        @blk.tensor
        def _(eng):
            eng.wait_ge(wv_sem, 16)  # weights resident
            for t in range(NT):
                acc = ps[t % NPS]
                if t >= NPS:
                    # bank's previous occupant (tile t-8, same parity) cast
                    eng.wait_ge(cast_sems[t % 2], (t - NPS) // 2 + 1)
                for g in range(G):
                    mm = nc.tensor.matmul(
                        acc[:],
                        wv_sb[:, 2 * g : 2 * g + 2, :],
                        s_sb[:, t, 2 * g : 2 * g + 2, :],
                        start=(g == 0), stop=(g == G - 1),
                        perf_mode=DR,
                    )
                    if g == 0:
                        mm._wait_ge(in_sems[t], 16)
                mm.then_inc(pe_sem, 1)

    return nc


def _scrub_debug_paths(nc):
    """Normalize per-instruction debug info (absolute file paths, tracebacks)
    so the serialized BIR is byte-identical regardless of where this file
    lives -- keeps the neuronxcc compile cache warm across directories."""
    import dataclasses

    def fix(obj):
        for attr in ("debug", "ant_debug"):
            dbg = getattr(obj, attr, None)
            if dbg is not None and getattr(dbg, "filename", None):
                setattr(
                    obj,
                    attr,
                    dataclasses.replace(
                        dbg, filename="kernel.py", ant_traceback=None
                    ),
                )

    for bb in nc.main_func.blocks:
        for ins in bb.instructions:
            fix(ins)
    for fn in nc.m.functions:
        for alloc in fn.allocations:
            fix(alloc)
            for ml in getattr(alloc, "memorylocations", None) or []:
                fix(ml)


def _get_nc():
    global _NC
    if _NC is None:
        import concourse.bass as bass

        _NC = _build_nc_fp8()
        _NC.compile()            # Bacc passes (reg alloc, sem gen, ...)
        _scrub_debug_paths(_NC)  # after compile so pass-inserted insts are hit
        bass.Bass.finalize(_NC)  # freeze (Bacc.finalize would re-run compile)
    return _NC


def prep_in_maps(s, means, var):
    import ml_dtypes

    f8np = ml_dtypes.float8_e4m3

    s = np.asarray(s, dtype=np.float32)
    means64 = np.asarray(means, dtype=np.float64)
    var64 = np.asarray(var, dtype=np.float64)

    inv = 1.0 / var64
    # W[d, k] = means[k, d] / var[d], packed as wv[p, c, k] with d = c*128 + p
    W = (means64 * inv[None, :]).T                          # (D, K)
    wv8 = np.ascontiguousarray(
        W.astype(np.float32).reshape(C, 128, K).transpose(1, 0, 2)
    ).astype(f8np)                                          # [p, c, k]

    # exact rank-1 terms, added on host during assembly
    log_norm = -0.5 * (D * np.log(2.0 * np.pi) + np.sum(np.log(var64)))
    m2 = (means64 * means64) @ inv                          # (K,)
    colvec = (-0.5 * m2).astype(np.float64)                 # (K,)
    s2 = (s.astype(np.float64) ** 2).reshape(-1, D) @ inv   # (B*T,)
    rowvec = (log_norm - 0.5 * s2).reshape(B, T)            # (B, T) fp64

    s8 = s.astype(f8np).reshape(N_CORES, NT, TN, C, 128)    # [i, t, n, c, p]
    in_maps = []
    for i in range(N_CORES):
        st_i = np.ascontiguousarray(s8[i].transpose(3, 0, 2, 1))  # [p,t,c,n]
        in_maps.append({"st": st_i, "wv": wv8})
    return in_maps, (rowvec, colvec)


def run_device(in_maps, trace=False, trace_kwargs=None):
    from concourse.bass_utils import run_bass_kernel_spmd

    return run_bass_kernel_spmd(
        _get_nc(),
        in_maps,
        list(range(N_CORES)),
        trace=trace,
        **(trace_kwargs or {}),
    )


def assemble(results, aux):
    rowvec, colvec = aux
    add = rowvec[:, :, None] + colvec[None, None, :]        # (B, T, K) fp64
    full = np.empty((B, T, K), dtype=np.float32)
    for i in range(N_CORES):
        o = np.asarray(results[i]["out"])                   # (K, R) int8
        full[i * BPC : (i + 1) * BPC] = (
            o.T.reshape(BPC, T, K).astype(np.float64) * OSCALE
            + add[i * BPC : (i + 1) * BPC]
        ).astype(np.float32)
    return full


def kernel(s, means, var):
    in_maps, aux = prep_in_maps(s, means, var)
    br = run_device(in_maps)
    return assemble(br.results, aux)


# revision 29
# speedup vs baseline: 1.0233x; 1.0233x over previous
"""DMVFlow per-state diagonal-Gaussian log-density kernel for 8 TRN2 NeuronCores.

density[b,t,k] = log_norm - 0.5*(s2[b,t] - 2*cross[b,t,k] + m2[k])
  with  log_norm = -0.5*(D*log(2pi) + sum_d log var[d])
        s2[b,t]  = sum_d s[b,t,d]^2 / var[d]
        cross    = sum_d s[b,t,d] * means[k,d] / var[d]
        m2[k]    = sum_d means[k,d]^2 / var[d]

Only cross[b,t,k] couples (b,t) with k; the per-row term (log_norm - 0.5*s2)
and per-state term (-0.5*m2) are rank-1 in the output and are computed exactly
on the host and added during assembly.  The device therefore runs a single
fp8(e4m3) GEMM per core: cross = s @ (means/var).T, using DoubleRow perf mode
(two 128-deep k-tiles per instruction).

Sharding: data-parallel over batch (32 sentences per core), weights replicated.

The kernel is input-DMA-bound (6.3 MB fp8 per core over 16 DMA engines at
~23 GB/s each).  Input arrives as st[p, t, c, n] = fp8(s[row = t*512 + n,
d = c*128 + p]) so DMA lines are contiguous per partition; tiles stream in
chunks of <=2 tiles (6KB packets -- 12KB packets measured ~20% slower/byte)
alternating across the sync and gpsimd queues so descriptor spin-up gaps on
one queue hide under the other queue's stream.  Device output is int8
(cross/2, |x| <= 118) to halve store traffic; host rescales and adds the
affine terms in fp32.  PSUM->int8 casts alternate DVE/ACT by tile parity.
"""

import numpy as np

N_CORES = 8
B, T, D, K = 256, 256, 768, 128
BPC = B // N_CORES          # batches per core
R = BPC * T                 # rows (token positions) per core = 8192
TN = 512                    # rows per tile (one PSUM bank)
NT = R // TN                # tiles per core = 16
C = D // 128                # contraction chunks = 6
G = C // 2                  # DoubleRow double-chunks = 3

OSCALE = 2.0                # host multiplier undoing the device's 0.5

# input DMA tiles per queue: 1-tile DMAs = 3KB packets, the measured
# per-DMA-engine sweet spot (21.5 GB/s; 6KB packets drop to 18, 12KB to
# 17); descriptor gaps on one queue hide under the other queue's stream.
# The two queues drain proportionally, so with a balanced split the last
# tiles of BOTH queues land bunched and the in-order PE serializes a
# 2-3 tile backlog.  Instead the scalar queue gets 7 tiles and drains
# early; sync then delivers 13,14,15 alone at full engine rate (~1.2us
# apart), which the PE consumes as they land.
SYNC_TILES = [0, 2, 4, 6, 8, 10, 13, 14, 15]
SCAL_TILES = [1, 3, 5, 7, 9, 11, 12]

# quad stores + even final tiles on gpsimd (DVE cannot issue DMAs);
# scalar stores odd final tiles 13/15 itself right after casting them,
# so the critical last store has zero cross-engine latency
STORES_GPS = [(0, 4), (4, 8), (8, 12), (12, 13), (14, 15)]
N_STORES = len(STORES_GPS) + 2

_NC = None                  # cached bass program (build once per process)


def _build_nc_fp8():
    """Hand-scheduled fp8 DoubleRow kernel: no TileContext, manual semaphores.

    Engine roles:
      sync   - weights DMA + even input tiles + tile 15 (HW-DGE queue)
      scalar - odd input tiles + tile 14 (HW-DGE queue), odd casts (ACT)
      gpsimd - output stores (own queue so they don't FIFO behind input)
      vector - even-tile PSUM casts (DVE)
      tensor - 3 DoubleRow matmuls per tile
    """
    from contextlib import ExitStack

    import concourse.bacc as bacc
    from concourse import mybir

    f8 = mybir.dt.float8e4
    i8 = mybir.dt.int8
    f32 = mybir.dt.float32
    DR = mybir.MatmulPerfMode.DoubleRow

    NPS = 8      # psum banks

    nc = bacc.Bacc(None, target_bir_lowering=False, debug=False)

    st = nc.dram_tensor("st", [128, NT, C, TN], f8, kind="ExternalInput")
    wv = nc.dram_tensor("wv", [128, C, K], f8, kind="ExternalInput")
    out = nc.dram_tensor("out", [K, R], i8, kind="ExternalOutput")

    with ExitStack() as ctx:
        e = ctx.enter_context
        s_sb = e(nc.sbuf_tensor([128, NT, C, TN], f8))
        o_sb = e(nc.sbuf_tensor([K, NT, TN], i8))
        wv_sb = e(nc.sbuf_tensor([128, C, K], f8))
        ps = [e(nc.psum_tensor(f"ps{i}", [K, TN], f32)) for i in range(NPS)]

        in_sems = [e(nc.semaphore(f"in{j}")) for j in range(NT)]
        wv_sem = e(nc.semaphore("wv_sem"))      # +16 when weights resident
        pe_sem = e(nc.semaphore("pe_sem"))      # +1 per finished MM group
        cast_sems = [e(nc.semaphore("castE")), e(nc.semaphore("castO"))]
        out_sem = e(nc.semaphore("out_sem"))    # +16 per completed store
        blk = e(nc.Block())

        def cast_wait(eng, lo, hi):
            # casts of tiles lo..hi-1 complete (per-parity counters)
            if hi - lo == 1:
                eng.wait_ge(cast_sems[lo % 2], lo // 2 + 1)
            else:
                eng.wait_ge(cast_sems[0], (hi + 1) // 2)
                eng.wait_ge(cast_sems[1], hi // 2)

        def issue_tiles(eng, tiles):
            for t in tiles:
                eng.dma_start(
                    s_sb[:, t, :, :], st[:, t, :, :]
                ).then_inc(in_sems[t], 16)

        @blk.sync
        def _(eng):
            eng.dma_start(wv_sb[:], wv[:]).then_inc(wv_sem, 16)
            issue_tiles(eng, SYNC_TILES)

        def store_tile(eng, t):
            eng.dma_start(
                out[:, t * TN : (t + 1) * TN], o_sb[:, t, :]
            ).then_inc(out_sem, 16)

        @blk.scalar
        def _(eng):
            issue_tiles(eng, SCAL_TILES)
            for t in range(1, NT, 2):
                eng.wait_ge(pe_sem, t + 1)
                nc.scalar.mul(o_sb[:, t, :], ps[t % NPS][:], 0.5).then_inc(
                    cast_sems[1], 1
                )
                if t >= 12:
                    store_tile(eng, t)

        @blk.gpsimd
        def _(eng):
            for lo, hi in STORES_GPS:
                cast_wait(eng, lo, hi)
                eng.dma_start(
                    out[:, lo * TN : hi * TN], o_sb[:, lo:hi, :]
                ).then_inc(out_sem, 16)
            eng.wait_ge(out_sem, 16 * N_STORES)

        @blk.vector
        def _(eng):
            for t in range(0, NT, 2):
                eng.wait_ge(pe_sem, t + 1)
                nc.vector.tensor_scalar_mul(
                    o_sb[:, t, :], ps[t % NPS][:], 0.5
                ).then_inc(cast_sems[0], 1)

        @blk.tensor
        def _(eng):
            eng.wait_ge(wv_sem, 16)  # weights resident
            for t in range(NT):
                acc = ps[t % NPS]
                if t >= NPS:
                    # bank's previous occupant (tile t-8, same parity) cast
                    eng.wait_ge(cast_sems[t % 2], (t - NPS) // 2 + 1)
                for g in range(G):
                    mm = nc.tensor.matmul(
                        acc[:],
                        wv_sb[:, 2 * g : 2 * g + 2, :],
                        s_sb[:, t, 2 * g : 2 * g + 2, :],
                        start=(g == 0), stop=(g == G - 1),
                        perf_mode=DR,
                    )
                    if g == 0:
                        mm._wait_ge(in_sems[t], 16)
                mm.then_inc(pe_sem, 1)

    return nc


def _scrub_debug_paths(nc):
    """Normalize per-instruction debug info (absolute file paths, tracebacks)
    so the serialized BIR is byte-identical regardless of where this file
    lives -- keeps the neuronxcc compile cache warm across directories."""
    import dataclasses

    def fix(obj):
        for attr in ("debug", "ant_debug"):
            dbg = getattr(obj, attr, None)
            if dbg is not None and getattr(dbg, "filename", None):
                setattr(
                    obj,
                    attr,
                    dataclasses.replace(
                        dbg, filename="kernel.py", ant_traceback=None
                    ),
                )

    for bb in nc.main_func.blocks:
        for ins in bb.instructions:
            fix(ins)
    for fn in nc.m.functions:
        for alloc in fn.allocations:
            fix(alloc)
            for ml in getattr(alloc, "memorylocations", None) or []:
                fix(ml)


def _get_nc():
    global _NC
    if _NC is None:
        import concourse.bass as bass

        _NC = _build_nc_fp8()
        _NC.compile()            # Bacc passes (reg alloc, sem gen, ...)
        _scrub_debug_paths(_NC)  # after compile so pass-inserted insts are hit
        bass.Bass.finalize(_NC)  # freeze (Bacc.finalize would re-run compile)
    return _NC


def prep_in_maps(s, means, var):
    import ml_dtypes

    f8np = ml_dtypes.float8_e4m3

    s = np.asarray(s, dtype=np.float32)
    means64 = np.asarray(means, dtype=np.float64)
    var64 = np.asarray(var, dtype=np.float64)

    inv = 1.0 / var64
    # W[d, k] = means[k, d] / var[d], packed as wv[p, c, k] with d = c*128 + p
    W = (means64 * inv[None, :]).T                          # (D, K)
    wv8 = np.ascontiguousarray(
        W.astype(np.float32).reshape(C, 128, K).transpose(1, 0, 2)
    ).astype(f8np)                                          # [p, c, k]

    # exact rank-1 terms, added on host during assembly
    log_norm = -0.5 * (D * np.log(2.0 * np.pi) + np.sum(np.log(var64)))
    m2 = (means64 * means64) @ inv                          # (K,)
    colvec = (-0.5 * m2).astype(np.float64)                 # (K,)
    s2 = (s.astype(np.float64) ** 2).reshape(-1, D) @ inv   # (B*T,)
    rowvec = (log_norm - 0.5 * s2).reshape(B, T)            # (B, T) fp64

    s8 = s.astype(f8np).reshape(N_CORES, NT, TN, C, 128)    # [i, t, n, c, p]
    in_maps = []
    for i in range(N_CORES):
        st_i = np.ascontiguousarray(s8[i].transpose(3, 0, 2, 1))  # [p,t,c,n]
        in_maps.append({"st": st_i, "wv": wv8})
    return in_maps, (rowvec, colvec)


def run_device(in_maps, trace=False, trace_kwargs=None):
    from concourse.bass_utils import run_bass_kernel_spmd

    return run_bass_kernel_spmd(
        _get_nc(),
        in_maps,
        list(range(N_CORES)),
        trace=trace,
        **(trace_kwargs or {}),
    )


def assemble(results, aux):
    rowvec, colvec = aux
    add = rowvec[:, :, None] + colvec[None, None, :]        # (B, T, K) fp64
    full = np.empty((B, T, K), dtype=np.float32)
    for i in range(N_CORES):
        o = np.asarray(results[i]["out"])                   # (K, R) int8
        full[i * BPC : (i + 1) * BPC] = (
            o.T.reshape(BPC, T, K).astype(np.float64) * OSCALE
            + add[i * BPC : (i + 1) * BPC]
        ).astype(np.float32)
    return full


def kernel(s, means, var):
    in_maps, aux = prep_in_maps(s, means, var)
    br = run_device(in_maps)
    return assemble(br.results, aux)
